# revision 2
# baseline (speedup 1.0000x reference)
"""BinaryDense forward on 8 Trainium2 NeuronCores.

Computes out = x @ (w_raw > 0) for x[4096,4096] f32, w_raw[4096,4096] f32.
(The straight-through-estimator forward is exactly the hard binary matmul.)

Sharding: 4 batch groups x 2 feature groups (one core each).
Per core: out.T[Nc=2048, Mc=1024] = (w_bin shard).T @ (x shard), K=4096.

Device kernel (per core):
  - stream x.T shard in 128-row k-chunks, split each f32 chunk into
    bf16 hi + bf16 lo tiles (cached in SBUF; hi+lo matmuls accumulated in
    PSUM give ~fp32 accuracy at bf16 PE throughput),
  - stream w_raw shard per (n-tile, k-chunk), binarize to bf16 {0,1} on DVE,
  - matmul with binarized w as the stationary operand (output transposed),
    accumulate 2*K/128 matmuls per PSUM bank, 8 banks in flight,
  - evict PSUM -> SBUF -> DRAM out.T.
"""

import numpy as np

_NCORES = 8
_G1 = 4  # batch groups
_G2 = 2  # feature groups
_B = 4096
_D = 4096
_F = 4096

_cache = {}
_DR_SCALE = 512.0


def _build(K, Mc, Nc, mode="split", repeat=1, opts=None):
    """Build + compile the per-core Bass program.

    xt: [K, Mc] f32 (x shard, transposed), wr: [K, Nc] f32 (w_raw shard),
    outT: [Nc, Mc] f32.
    """
    import concourse.bacc as bacc
    import concourse.tile as tile
    from concourse import mybir

    opts = dict(opts or {})
    wbufs = opts.get("wbufs", 4)
    evbufs = opts.get("evbufs", 4)
    xsbufs = opts.get("xsbufs", 3)
    ev_eng = opts.get("ev_eng", "vector")

    dt = mybir.dt
    P = 128
    NT = 512  # n-tile (psum free dim)
    MC = 512  # m moving chunk
    KC = K // P
    NTC = Nc // NT
    MCC = Mc // MC
    NNC = NT // P  # stationary 128-slices per n-tile

    nc = bacc.Bacc(None, target_bir_lowering=False, debug=False, num_devices=_NCORES)

    xt_d = nc.dram_tensor("xt", [K, Mc], dt.float32, kind="ExternalInput")
    wr_d = nc.dram_tensor("wr", [K, Nc], dt.float32, kind="ExternalInput")
    outT_d = nc.dram_tensor("outT", [Nc, Mc], dt.float32, kind="ExternalOutput")

    with tile.TileContext(nc) as tc:
        with (
            tc.tile_pool(name="xcache", bufs=1) as xcache,
            tc.tile_pool(name="xstage", bufs=xsbufs) as xstage,
            tc.tile_pool(name="wpool", bufs=wbufs) as wpool,
            tc.tile_pool(name="evpool", bufs=evbufs) as evpool,
            tc.tile_pool(name="pspool", bufs=1, space="PSUM") as pspool,
        ):
            def emit_xprep(k, xparts, xdrs):
                # x chunk k: DMA f32 -> split to bf16 hi + lo (or cast)
                xs = xstage.tile([P, Mc], dt.float32, name="xs", tag="xs")
                nc.sync.dma_start(xs[:], xt_d[k * P : (k + 1) * P, :])
                if mode in ("split", "splith"):
                    hdt = dt.bfloat16 if mode == "split" else dt.float16
                    xhi = xcache.tile([P, Mc], hdt, name=f"xhi{k}", tag=f"xhi{k}")
                    nc.scalar.copy(xhi[:], xs[:])
                    xlo = xcache.tile([P, Mc], hdt, name=f"xlo{k}", tag=f"xlo{k}")
                    nc.vector.tensor_sub(xlo[:], xs[:], xhi[:])
                    xparts.append((xhi, xlo))
                elif mode == "drsplit":
                    xhi = xcache.tile([P, Mc], dt.bfloat16, name=f"xhi{k}", tag=f"xhi{k}")
                    nc.scalar.copy(xhi[:], xs[:])
                    xlo = xstage.tile([P, Mc], dt.bfloat16, name="xlo", tag="xlo")
                    nc.vector.tensor_sub(xlo[:], xs[:], xhi[:])
                    # pack scaled fp8 residual into DR plane k%2 of chunk k//2
                    xparts.append(xhi)
                    k2 = k // 2
                    if k % 2 == 0:
                        xdr = xcache.tile(
                            [P, 2, Mc], dt.float8e4, name=f"xdr{k2}", tag=f"xdr{k2}"
                        )
                        xdrs.append(xdr)
                    nc.vector.tensor_scalar_mul(
                        xdrs[k2][:, k % 2, :], xlo[:], _DR_SCALE
                    )
                elif mode == "f32r":
                    xr = xcache.tile([P, Mc], dt.float32r, name=f"xr{k}", tag=f"xr{k}")
                    nc.vector.tensor_copy(xr[:], xs[:])
                    xparts.append((xr,))
                elif mode == "bf16":
                    xhi = xcache.tile([P, Mc], dt.bfloat16, name=f"xhi{k}", tag=f"xhi{k}")
                    nc.scalar.copy(xhi[:], xs[:])
                    xparts.append((xhi,))
                else:
                    raise ValueError(mode)

            def emit_fdr_body():
                # Fused hi/lo fp8 DoubleRow: x = fp8(x) + fp8(residual), both
                # unscaled, so the hi and lo matmuls accumulate into the same
                # PSUM bank. DR planes pack adjacent 128-row k-chunks, so the
                # binarized w needs no duplication. 1024 DR matmuls/iter.
                KC2 = KC // 2
                xhis, xlos = [], []
                for k in range(KC):
                    k2, kp = k // 2, k % 2
                    if kp == 0:
                        xhi = xcache.tile(
                            [P, 2, Mc], dt.float8e4, name=f"xhi{k2}",
                            tag=f"xhi{k2}", bufs=2,
                        )
                        xlo = xcache.tile(
                            [P, 2, Mc], dt.float8e4, name=f"xlo{k2}",
                            tag=f"xlo{k2}", bufs=2,
                        )
                        xhis.append(xhi)
                        xlos.append(xlo)
                    xs = xstage.tile([P, Mc], dt.float32, name="xs", tag="xs")
                    nc.sync.dma_start(xs[:], xt_d[k * P : (k + 1) * P, :])
                    nc.scalar.copy(xhis[k2][:, kp, :], xs[:])
                    nc.vector.tensor_sub(xlos[k2][:, kp, :], xs[:], xhis[k2][:, kp, :])
                for nt in range(NTC):
                    psums = {}
                    for nn in range(NNC):
                        for mc in range(MCC):
                            psums[(nn, mc)] = pspool.tile(
                                [P, MC], dt.float32,
                                name=f"ps{nn}_{mc}", tag=f"ps{nn}_{mc}",
                            )
                    for k2 in range(KC2):
                        wdr = wpool.tile(
                            [P, 2, NT], dt.float8e4, name="wdr", tag="wdr"
                        )
                        for kp in range(2):
                            k = 2 * k2 + kp
                            wf = wpool.tile([P, NT], dt.float32, name="wf", tag="wf")
                            nc.sync.dma_start(
                                wf[:],
                                wr_d[k * P : (k + 1) * P, nt * NT : (nt + 1) * NT],
                            )
                            nc.vector.tensor_scalar(
                                wdr[:, kp, :], wf[:], 0.0, None, mybir.AluOpType.is_gt
                            )
                        for nn in range(NNC):
                            for pi, xsrc in enumerate((xhis, xlos)):
                                for mc in range(MCC):
                                    nc.tensor.matmul(
                                        psums[(nn, mc)][:],
                                        wdr[:, :, nn * P : (nn + 1) * P],
                                        xsrc[k2][:, :, mc * MC : (mc + 1) * MC],
                                        start=(k2 == 0 and pi == 0),
                                        stop=(k2 == KC2 - 1 and pi == 1),
                                        perf_mode=mybir.MatmulPerfMode.DoubleRow,
                                    )
                    for nn in range(NNC):
                        for mc in range(MCC):
                            ev = evpool.tile([P, MC], dt.float32, name="ev", tag="ev")
                            nc.scalar.copy(ev[:], psums[(nn, mc)][:])
                            nc.sync.dma_start(
                                outT_d[
                                    nt * NT + nn * P : nt * NT + (nn + 1) * P,
                                    mc * MC : (mc + 1) * MC,
                                ],
                                ev[:],
                            )

            def emit_body():
                if mode == "fdr":
                    return emit_fdr_body()
                xparts = []
                xdrs = []
                for k in range(KC):
                    emit_xprep(k, xparts, xdrs)
                wdt = {
                    "split": dt.bfloat16,
                    "splith": dt.float16,
                    "bf16": dt.bfloat16,
                    "f32r": dt.float32r,
                    "drsplit": dt.bfloat16,
                }[mode]

                if mode == "drsplit":
                    for nt in range(NTC):
                        psums = {}
                        for nn in range(NNC):
                            for mc in range(MCC):
                                psums[(nn, mc)] = pspool.tile(
                                    [P, MC], dt.float32,
                                    name=f"ps{nn}_{mc}", tag=f"ps{nn}_{mc}",
                                )
                        wdrs = {}
                        # hi pass: bf16, K chunks of 128
                        for k in range(KC):
                            wf = wpool.tile([P, NT], dt.float32, name="wf", tag="wf")
                            nc.sync.dma_start(
                                wf[:], wr_d[k * P : (k + 1) * P, nt * NT : (nt + 1) * NT]
                            )
                            wb = wpool.tile([P, NT], dt.bfloat16, name="wb", tag="wb")
                            nc.vector.tensor_scalar(
                                wb[:], wf[:], 0.0, None, mybir.AluOpType.is_gt
                            )
                            # also binarize into the fp8 DR plane for the lo pass
                            k2, kp = k // 2, k % 2
                            if kp == 0:
                                wdrs[k2] = wpool.tile(
                                    [P, 2, NT], dt.float8e4,
                                    name="wdr", tag="wdr", bufs=KC // 2 + 2,
                                )
                            nc.vector.tensor_scalar(
                                wdrs[k2][:, kp, :], wf[:], 0.0, None,
                                mybir.AluOpType.is_gt,
                            )
                            for nn in range(NNC):
                                for mc in range(MCC):
                                    nc.tensor.matmul(
                                        psums[(nn, mc)][:],
                                        wb[:, nn * P : (nn + 1) * P],
                                        xparts[k][:, mc * MC : (mc + 1) * MC],
                                        start=(k == 0),
                                        stop=(k == KC - 1),
                                    )
                        # evict hi results to SBUF, then lo pass reuses banks
                        hiparts = {}
                        for nn in range(NNC):
                            for mc in range(MCC):
                                hv = evpool.tile(
                                    [P, MC], dt.float32,
                                    name=f"hi{nn}_{mc}", tag=f"hi{nn}_{mc}", bufs=2,
                                )
                                nc.scalar.copy(hv[:], psums[(nn, mc)][:])
                                hiparts[(nn, mc)] = hv
                        psums2 = {}
                        for nn in range(NNC):
                            for mc in range(MCC):
                                psums2[(nn, mc)] = pspool.tile(
                                    [P, MC], dt.float32,
                                    name=f"ps{nn}_{mc}", tag=f"ps{nn}_{mc}",
                                )
                        for k2 in range(KC // 2):
                            for nn in range(NNC):
                                for mc in range(MCC):
                                    nc.tensor.matmul(
                                        psums2[(nn, mc)][:],
                                        wdrs[k2][:, :, nn * P : (nn + 1) * P],
                                        xdrs[k2][:, :, mc * MC : (mc + 1) * MC],
                                        start=(k2 == 0),
                                        stop=(k2 == KC // 2 - 1),
                                        perf_mode=mybir.MatmulPerfMode.DoubleRow,
                                    )
                        for nn in range(NNC):
                            for mc in range(MCC):
                                ev = evpool.tile([P, MC], dt.float32, name="ev", tag="ev")
                                nc.vector.scalar_tensor_tensor(
                                    ev[:], psums2[(nn, mc)][:], 1.0 / _DR_SCALE,
                                    hiparts[(nn, mc)][:],
                                    mybir.AluOpType.mult, mybir.AluOpType.add,
                                )
                                nc.sync.dma_start(
                                    outT_d[
                                        nt * NT + nn * P : nt * NT + (nn + 1) * P,
                                        mc * MC : (mc + 1) * MC,
                                    ],
                                    ev[:],
                                )
                    return

                # ---- main: per n-tile, accumulate over k into 8 psum banks ----
                for nt in range(NTC):
                    psums = {}
                    for nn in range(NNC):
                        for mc in range(MCC):
                            psums[(nn, mc)] = pspool.tile(
                                [P, MC],
                                dt.float32,
                                name=f"ps{nn}_{mc}",
                                tag=f"ps{nn}_{mc}",
                            )
                    nparts = len(xparts[0])
                    for k in range(KC):
                        wf = wpool.tile([P, NT], dt.float32, name="wf", tag="wf")
                        nc.sync.dma_start(
                            wf[:], wr_d[k * P : (k + 1) * P, nt * NT : (nt + 1) * NT]
                        )
                        wb = wpool.tile([P, NT], wdt, name="wb", tag="wb")
                        nc.vector.tensor_scalar(
                            wb[:], wf[:], 0.0, None, mybir.AluOpType.is_gt
                        )
                        for nn in range(NNC):
                            for pi in range(nparts):
                                for mc in range(MCC):
                                    nc.tensor.matmul(
                                        psums[(nn, mc)][:],
                                        wb[:, nn * P : (nn + 1) * P],
                                        xparts[k][pi][:, mc * MC : (mc + 1) * MC],
                                        start=(k == 0 and pi == 0),
                                        stop=(k == KC - 1 and pi == nparts - 1),
                                    )
                    for nn in range(NNC):
                        for mc in range(MCC):
                            ev = evpool.tile([P, MC], dt.float32, name="ev", tag="ev")
                            if ev_eng == "vector":
                                nc.vector.tensor_copy(ev[:], psums[(nn, mc)][:])
                            else:
                                nc.scalar.copy(ev[:], psums[(nn, mc)][:])
                            nc.sync.dma_start(
                                outT_d[
                                    nt * NT + nn * P : nt * NT + (nn + 1) * P,
                                    mc * MC : (mc + 1) * MC,
                                ],
                                ev[:],
                            )

            if repeat == 1:
                emit_body()
            elif opts.get("unroll"):
                for _ in range(repeat):
                    emit_body()
            else:
                with tc.For_i(0, repeat, 1):
                    emit_body()

    nc.compile()
    return nc


def _get_nc(K, Mc, Nc, mode="split", repeat=1, opts=None):
    key = (K, Mc, Nc, mode, repeat, tuple(sorted((opts or {}).items())))
    if key not in _cache:
        _cache[key] = _build(K, Mc, Nc, mode, repeat, opts)
    return _cache[key]


def _run(x, w_raw, mode="split", repeat=1):
    """Shard, run on 8 cores, gather. x:[B,D] f32, w_raw:[D,F] f32."""
    from concourse.bass_utils import run_bass_kernel_spmd

    B, D = x.shape
    D2, F = w_raw.shape
    assert D == D2
    Mc = B // _G1
    Nc = F // _G2

    nc = _get_nc(D, Mc, Nc, mode, repeat)

    xt = np.ascontiguousarray(x.T)  # [D, B]
    in_maps = []
    for c in range(_NCORES):
        i, j = c // _G2, c % _G2
        in_maps.append(
            {
                "xt": np.ascontiguousarray(xt[:, i * Mc : (i + 1) * Mc]),
                "wr": np.ascontiguousarray(w_raw[:, j * Nc : (j + 1) * Nc]),
            }
        )

    res = run_bass_kernel_spmd(nc, in_maps, list(range(_NCORES)))

    outT = np.empty((F, B), dtype=np.float32)
    for c in range(_NCORES):
        i, j = c // _G2, c % _G2
        outT[j * Nc : (j + 1) * Nc, i * Mc : (i + 1) * Mc] = res.results[c]["outT"]
    return np.ascontiguousarray(outT.T)


def kernel(x, w_raw):
    x = np.asarray(x, dtype=np.float32)
    w_raw = np.asarray(w_raw, dtype=np.float32)
    return _run(x, w_raw, mode="split", repeat=1)

